# revision 53
# baseline (speedup 1.0000x reference)
"""Multi-head attention (B=2, S=2048, H=1024, NH=16, DK=DV=64) on 8 TRN2 cores.

Sharding: data-parallel over batch (2 groups of 4 cores) x tensor-parallel
over heads (4 heads per core).  Each core computes, for its batch sample and
its 4 heads:
    Q^T/K^T projections (features on partitions), V projection (natural),
    S^T = K @ Q^T per 128-key chunk (causal chunks only; the two heads of a
    pair run as concurrent row-tiled K=64 matmuls into one 2-bank PSUM tile),
    P^T = exp(S^T/8 + pad_bias)  (one ACTIVATE covers both heads),
    out^T = V_aug^T @ P^T  where V_aug = [V | ones] for even heads and
    [ones | V] for odd heads, so values and 1/denominator stay
    partition-aligned for both halves of attn^T,
    y_partial = attn^T.T @ W_O_rows   (row-sharded W_O).
Host sums the 4 bf16 partials per batch and adds b_V @ W_O + b_O (exact
fold of the V bias through the output projection).

The emission is hand-staged so the ACT engine (exp is the serial bottleneck,
~88us/core) starts ~12us in and never starves, while projection/output
matmuls fill the PE between attention chunks and keep the PE HAM-warm:

    S1   : pair-0 Q/K projections as an 8-bank PSUM wave (c-outer), paced by
           the x^T DMA stream
    S2-5 : per j: V-projection chunks for j's keys, then att(j, pair0)
    S6-7 : pair-1 Q/K projection groups woven between att(0..1, pair1)
    S8-9 : att(2..3, pair1) with W_O(0..2) woven into the chunk loops
    S10  : W_O(3)

Everything on the wide data path is bf16 (fp32 accumulation in PSUM).
Engine balance: PE matmuls only; ACT exps only; DVE does projection
writebacks, normalization, and y casts; GpSimd does the causal-diagonal
masks.  All PSUM pool scopes are arranged to stay within the 8 banks.
"""

import math
from contextlib import ExitStack

import numpy as np

import concourse.bass as bass
import concourse.mybir as mybir
from concourse import bacc
import concourse.tile as tile
from concourse.bass_utils import run_bass_kernel_spmd

F32 = mybir.dt.float32
BF16 = mybir.dt.bfloat16
EXP = mybir.ActivationFunctionType.Exp

B, S, H = 2, 2048, 1024
NH, DK, DV = 16, 64, 64
NCORE = 8
NCH = H // 128          # 8 contraction chunks over H
NJ = S // 512           # 4 query subtiles of 512
NKC = S // 128          # 16 key chunks
NPAIR = 2               # head pairs per core
SCALE = 1.0 / math.sqrt(DK)
NEG_BIAS = -30000.0     # exp(x + NEG_BIAS) == 0.0 in fp32 for any real score


def _emit(nc, d):
    with tile.TileContext(nc) as tc, ExitStack() as top:
        consts = top.enter_context(tc.tile_pool(name="consts", bufs=1))
        persist = top.enter_context(tc.tile_pool(name="persist", bufs=1))
        xtp = top.enter_context(tc.tile_pool(name="xtp", bufs=1))

        # ---- persistent activations ----
        qt_sb = []   # per pair: [128, S] bf16; rows 0:64 head A, 64:128 head B
        kt_sb = []
        attnT = []   # per pair: [128, S] bf16 normalized attn^T
        for p in range(NPAIR):
            qt_sb.append(persist.tile([128, S], BF16, tag=f"qt{p}", name=f"qt{p}sb"))
            kt_sb.append(persist.tile([128, S], BF16, tag=f"kt{p}", name=f"kt{p}sb"))
            attnT.append(persist.tile([128, S], BF16, tag=f"at{p}", name=f"at{p}sb"))
        # V_aug per head: [128 keys, NKC*128]; chunk t block is [V|ones] for
        # even heads, [ones|V] for odd heads.
        vaug = []
        for h in range(4):
            v = persist.tile([128, NKC * 128], BF16, tag=f"vaug{h}", name=f"vaug{h}sb")
            nc.vector.memset(v, 1.0)
            vaug.append(v)

        xt_sb = [xtp.tile([128, S], BF16, tag=f"xt{c}", name=f"xt{c}sb")
                 for c in range(NCH)]

        # Two DMA rings: sync carries pair-0 weights + the x^T stream (the
        # critical path to first matmul); the ACT ring carries everything
        # needed later, in parallel.
        wqq_sb = []
        wkk_sb = []
        for p in range(NPAIR):
            wqq_sb.append(consts.tile([128, NCH * 128], BF16, tag=f"wqq{p}",
                                      name=f"wqq{p}sb"))
            wkk_sb.append(consts.tile([128, NCH * 128], BF16, tag=f"wkk{p}",
                                      name=f"wkk{p}sb"))
        # sync ring: nothing but the x^T stream (the gate for every matmul);
        # scalar ring: all weights, pair-0 first.
        for c in range(NCH):
            nc.sync.dma_start(out=xt_sb[c], in_=d["xt"][c * 128:(c + 1) * 128, :])
        nc.scalar.dma_start(out=wqq_sb[0], in_=d["wqq"][0])
        nc.scalar.dma_start(out=wkk_sb[0], in_=d["wkk"][0])
        bq_sb = consts.tile([128, 2], F32, tag="bq", name="bqsb")
        nc.scalar.dma_start(out=bq_sb, in_=d["bq"][:])
        bk_sb = consts.tile([128, 2], F32, tag="bk", name="bksb")
        nc.scalar.dma_start(out=bk_sb, in_=d["bk"][:])
        nbias_sb = consts.tile([128, NKC], F32, tag="nbias", name="nbiassb")
        nc.scalar.dma_start(out=nbias_sb, in_=d["nbias"][:])
        wv_sb = consts.tile([128, NCH * 256], BF16, tag="wv", name="wvsb")
        nc.scalar.dma_start(out=wv_sb, in_=d["wv"][:])
        nc.scalar.dma_start(out=wqq_sb[1], in_=d["wqq"][1])
        nc.scalar.dma_start(out=wkk_sb[1], in_=d["wkk"][1])
        wo_sb = consts.tile([128, 2 * 1024], BF16, tag="wo", name="wosb")
        nc.scalar.dma_start(out=wo_sb, in_=d["wo"][:])
        mdiag_sb = consts.tile([128, 2, 128], BF16, tag="mdiag", name="mdiagsb")
        nc.gpsimd.dma_start(out=mdiag_sb, in_=d["mdiag"][:])

        # ---- S1: pair-0 Q/K projections, 8-bank wave paced by the xt DMA ----
        with tc.tile_pool(name="psqk8", bufs=1, space="PSUM") as psqk8:
            pss = {}
            for qk in range(2):
                for j in range(NJ):
                    pss[qk, j] = psqk8.tile([128, 512], F32, tag=f"pp{qk}{j}",
                                            name=f"pp{qk}{j}")
            for c in range(NCH - 1):
                for qk, wsb in ((0, wqq_sb[0]), (1, wkk_sb[0])):
                    for j in range(NJ):
                        nc.tensor.matmul(
                            pss[qk, j],
                            wsb[:, c * 128:(c + 1) * 128],
                            xt_sb[c][:, j * 512:(j + 1) * 512],
                            start=(c == 0), stop=False,
                        )
            # last chunk tile-major with immediate writeback so the 8
            # writebacks pipeline with the remaining matmuls instead of
            # draining serially at the end (j-major: att(0,0) unblocks first)
            c = NCH - 1
            for j in range(NJ):
                for qk, (wsb, bsb, dst) in enumerate((
                        (wqq_sb[0], bq_sb, qt_sb[0]),
                        (wkk_sb[0], bk_sb, kt_sb[0]))):
                    nc.tensor.matmul(
                        pss[qk, j],
                        wsb[:, c * 128:(c + 1) * 128],
                        xt_sb[c][:, j * 512:(j + 1) * 512],
                        start=False, stop=True,
                    )
                    nc.scalar.activation(
                        dst[:, j * 512:(j + 1) * 512], pss[qk, j],
                        mybir.ActivationFunctionType.Identity,
                        bias=bsb[:, 0:1])

        with (
            tc.tile_pool(name="psab", bufs=2, space="PSUM") as psab,
            tc.tile_pool(name="pso", bufs=1, space="PSUM") as psop,
            tc.tile_pool(name="ptp", bufs=4) as ptp,
            tc.tile_pool(name="nrm", bufs=3) as nrm,
            tc.tile_pool(name="ysb", bufs=4) as ysb,
        ):
            def qkproj_group(pool, p, qk, j):
                """Pair-1 style single-tile projection group (c-inner)."""
                wsb = (wqq_sb, wkk_sb)[qk][p]
                bsb = (bq_sb, bk_sb)[qk]
                dst = (qt_sb, kt_sb)[qk][p]
                ps = pool.tile([128, 512], F32, tag="sc", name=f"qk{p}{qk}{j}")
                for c in range(NCH):
                    nc.tensor.matmul(
                        ps, wsb[:, c * 128:(c + 1) * 128],
                        xt_sb[c][:, j * 512:(j + 1) * 512],
                        start=(c == 0), stop=(c == NCH - 1),
                    )
                # writeback on ACT: it slots between exps right where the
                # dependent scores need it, with no DVE-queue latency
                nc.scalar.activation(
                    dst[:, j * 512:(j + 1) * 512], ps,
                    mybir.ActivationFunctionType.Identity,
                    bias=bsb[:, p:p + 1])

            def vproj(pool, t):
                ps = pool.tile([128, 256], F32, tag="v", name=f"psv{t}")
                for c in range(NCH):
                    nc.tensor.matmul(
                        ps, xt_sb[c][:, t * 128:(t + 1) * 128],
                        wv_sb[:, c * 256:(c + 1) * 256],
                        start=(c == 0), stop=(c == NCH - 1),
                    )
                for h in range(4):
                    off = t * 128 + (0 if h % 2 == 0 else 64)
                    nc.vector.tensor_copy(vaug[h][:, off:off + 64],
                                          ps[:, h * 64:(h + 1) * 64])

            def norm_cols(p, j, oA, oB, scr, rec, col0, col1, on_act):
                """Normalize columns [col0:col1) of the (j,p) tile.  Legal
                before the accumulation group ends: PV chunk t only writes
                columns 128t onward, so lower columns are final early."""
                csl = slice(col0, col1)
                jsl = slice(j * 512 + col0, j * 512 + col1)
                if on_act:
                    nc.scalar.activation(scr[64:128, csl], oA[64:128, csl],
                                         mybir.ActivationFunctionType.Copy)
                    nc.scalar.activation(scr[0:64, csl], oB[0:64, csl],
                                         mybir.ActivationFunctionType.Copy)
                else:
                    nc.vector.tensor_copy(scr[64:128, csl], oA[64:128, csl])
                    nc.vector.tensor_copy(scr[0:64, csl], oB[0:64, csl])
                nc.sync.dma_start(out=rec[0:64, csl], in_=scr[64:128, csl])
                nc.sync.dma_start(out=rec[64:128, csl], in_=scr[0:64, csl])
                nc.vector.reciprocal_approx_fast(out=rec[:, csl],
                                                 in_=rec[:, csl])
                nc.vector.tensor_mul(attnT[p][0:64, jsl], oA[0:64, csl],
                                     rec[0:64, csl])
                nc.vector.tensor_mul(attnT[p][64:128, jsl], oB[64:128, csl],
                                     rec[64:128, csl])

            def finish_norm(j, p, oA, oB, last):
                # normalize: head A denom at oA[64:128], head B denom at
                # oB[0:64]; one cross DMA each makes values/recip lane-aligned.
                jsl = slice(j * 512, (j + 1) * 512)
                scr = nrm.tile([128, 512], F32, tag="scr", name=f"scr{p}{j}")
                if last:
                    nc.scalar.activation(scr[64:128, :], oA[64:128, :],
                                         mybir.ActivationFunctionType.Copy)
                    nc.scalar.activation(scr[0:64, :], oB[0:64, :],
                                         mybir.ActivationFunctionType.Copy)
                else:
                    nc.vector.tensor_copy(scr[64:128, :], oA[64:128, :])
                    nc.vector.tensor_copy(scr[0:64, :], oB[0:64, :])
                rec = nrm.tile([128, 512], F32, tag="rec", name=f"rec{p}{j}")
                nc.sync.dma_start(out=rec[0:64, :], in_=scr[64:128, :])
                nc.sync.dma_start(out=rec[64:128, :], in_=scr[0:64, :])
                nc.vector.reciprocal_approx_fast(out=rec, in_=rec)
                nc.vector.tensor_mul(attnT[p][0:64, jsl], oA[0:64, :],
                                     rec[0:64, :])
                nc.vector.tensor_mul(attnT[p][64:128, jsl], oB[64:128, :],
                                     rec[64:128, :])

            def att(j, p, weave=None, last=False, stage_wo=None):
                """Attention for (j, pair p): scores/exp/mask/PV + normalize.
                weave: optional {chunk_index: fn} emitted after that chunk.
                last: put the denominator copies on ACT (idle at the tail)
                to shorten the final normalize->W_O critical path."""
                hA, hB = 2 * p, 2 * p + 1
                oA = psop.tile([128, 512], F32, tag="oA", bufs=1, name=f"oA{p}{j}")
                oB = psop.tile([128, 512], F32, tag="oB", bufs=1, name=f"oB{p}{j}")
                if stage_wo:
                    s_scr = nrm.tile([128, 512], F32, tag="scr", name=f"sscr{p}{j}")
                    s_rec = nrm.tile([128, 512], F32, tag="rec", name=f"srec{p}{j}")
                cmax = 4 * j + 3
                for c in range(cmax + 1):
                    t = c - 4 * j
                    fo = 128 * t if t > 0 else 0
                    w = 512 - fo
                    qsl = slice(j * 512 + fo, (j + 1) * 512)
                    ksl = slice(c * 128, (c + 1) * 128)
                    sAB = psab.tile([128, 2, 512], F32, tag="sAB",
                                    name=f"sAB{p}{j}{c}")
                    nc.tensor.matmul(
                        sAB[:, 0, :w], kt_sb[p][0:64, ksl],
                        qt_sb[p][0:64, qsl], start=True, stop=True)
                    nc.tensor.matmul(
                        sAB[:, 1, :w], kt_sb[p][64:128, ksl],
                        qt_sb[p][64:128, qsl], start=True, stop=True)
                    pAB = ptp.tile([128, 2, 512], BF16, tag="pAB",
                                   name=f"pAB{p}{j}{c}")
                    if w == 512:
                        nc.scalar.activation(pAB[:, :, :], sAB[:, :, :], EXP,
                                             bias=nbias_sb[:, c:c + 1],
                                             scale=SCALE)
                    else:
                        nc.scalar.activation(pAB[:, :, :w], sAB[:, :, :w], EXP,
                                             bias=nbias_sb[:, c:c + 1],
                                             scale=SCALE)
                    if t >= 0:
                        # diagonal 128x128 block: zero keys below the diagonal
                        nc.gpsimd.tensor_mul(pAB[:, :, 0:128],
                                             pAB[:, :, 0:128], mdiag_sb)
                    nc.tensor.matmul(
                        oA[:, fo:512], vaug[hA][:, ksl], pAB[:, 0, :w],
                        start=(c == 0), stop=(c == cmax))
                    nc.tensor.matmul(
                        oB[:, fo:512], vaug[hB][:, ksl], pAB[:, 1, :w],
                        start=(c == 0), stop=(c == cmax))
                    if weave and c in weave:
                        weave[c]()
                    if stage_wo and t >= 0:
                        # columns [128t:128t+128) got their last PV write at
                        # this chunk — normalize them and run their W_O now
                        norm_cols(p, j, oA, oB, s_scr, s_rec,
                                  128 * t, 128 * t + 128, on_act=(t == 3))
                        stage_wo(t)
                if stage_wo:
                    return
                finish_norm(j, p, oA, oB, last)

            def wo_tile(pool, q, half, cast_eng=None, pairs=(0, 1), ydram=None,
                        yrow=None):
                pf = pool.tile([128, 512], F32, tag="sc", name=f"pf{q}{half}")
                for i, p in enumerate(pairs):
                    nc.tensor.matmul(
                        pf, attnT[p][:, q * 128:(q + 1) * 128],
                        wo_sb[:, p * 1024 + half * 512:
                              p * 1024 + half * 512 + 512],
                        start=(i == 0), stop=(i == len(pairs) - 1),
                    )
                yt = ysb.tile([128, 512], BF16, tag="y", name=f"yt{q}{half}")
                if cast_eng == "scalar":
                    nc.scalar.activation(yt, pf,
                                         mybir.ActivationFunctionType.Copy)
                else:
                    nc.vector.tensor_copy(yt, pf)
                ydst = d["y"] if ydram is None else d[ydram]
                row = q * 128 if yrow is None else yrow
                nc.sync.dma_start(
                    out=ydst[row:row + 128, half * 512:(half + 1) * 512],
                    in_=yt)

            # ---- S2-S5: V projection woven into att(*, pair0) chunk loops ----
            with tc.tile_pool(name="psv", bufs=2, space="PSUM") as psv:
                for t in range(4):
                    vproj(psv, t)
                att(0, 0)
                for j in range(1, NJ):
                    att(j, 0, weave={i: (lambda t=4 * j + i: vproj(psv, t))
                                     for i in range(4)})

            # ---- S6-S10: pair-1 projections + W_O woven into att(*, pair1);
            # qk-group and W_O psum tiles share one 2-bank rotating pool ----
            with tc.tile_pool(name="ps2", bufs=2, space="PSUM") as ps2:
                def qkg(qk, j):
                    return lambda: qkproj_group(ps2, 1, qk, j)

                def wot(q, half, cast_eng=None):
                    return lambda: wo_tile(ps2, q, half, cast_eng)

                qkproj_group(ps2, 1, 0, 0)
                qkproj_group(ps2, 1, 1, 0)
                att(0, 1, weave={0: qkg(0, 1), 2: qkg(1, 1)})
                # WO tiles woven one per chunk, each a stage behind its
                # normalize so the PE never waits on the DVE chain
                att(1, 1, weave={1: wot(0, 0), 2: wot(0, 1), 3: qkg(0, 2),
                                 4: wot(1, 0), 5: qkg(1, 2), 6: wot(1, 1),
                                 7: wot(2, 0)})
                att(2, 1, weave={0: wot(2, 1), 1: wot(3, 0), 2: qkg(0, 3),
                                 3: wot(3, 1), 4: wot(4, 0), 6: qkg(1, 3),
                                 7: wot(4, 1), 8: wot(5, 0), 9: wot(5, 1),
                                 10: wot(6, 0), 11: wot(6, 1)})
                # att(3,1) also absorbs the pair-0 halves of W_O(3) (split
                # output: y2 holds attnT0@W_O0 for rows 1536:2048, summed on
                # the host), so the tail after the last normalize is only the
                # eight single-matmul pair-1 tiles.
                def wop0(q, half):
                    return lambda: wo_tile(ps2, q, half, None, pairs=(0,),
                                           ydram="y2", yrow=(q - 12) * 128)

                def two(f, g):
                    return lambda: (f(), g())

                att(3, 1, weave={0: wot(7, 0), 1: wot(7, 1),
                                 2: wop0(12, 0), 3: wop0(12, 1),
                                 4: wot(8, 0), 5: wot(8, 1),
                                 6: wop0(13, 0), 7: wot(9, 0),
                                 8: wot(9, 1), 9: wop0(13, 1),
                                 10: wot(10, 0), 11: wot(10, 1),
                                 12: two(wop0(14, 0), wop0(14, 1)),
                                 13: wot(11, 0), 14: wot(11, 1),
                                 15: two(wop0(15, 0), wop0(15, 1))},
                    last=True)
                for q in range(12, 16):
                    wo_tile(ps2, q, 0, "scalar", pairs=(1,))
                    wo_tile(ps2, q, 1, None, pairs=(1,))


_NC_CACHE = {}


def _get_nc():
    if "nc" not in _NC_CACHE:
        nc = bacc.Bacc(None, target_bir_lowering=False)
        d = {
            "xt": nc.dram_tensor("xt", [H, S], BF16, kind="ExternalInput"),
            "wqq": nc.dram_tensor("wqq", [NPAIR, 128, NCH * 128], BF16,
                                  kind="ExternalInput"),
            "wkk": nc.dram_tensor("wkk", [NPAIR, 128, NCH * 128], BF16,
                                  kind="ExternalInput"),
            "wv": nc.dram_tensor("wv", [128, NCH * 256], BF16,
                                 kind="ExternalInput"),
            "wo": nc.dram_tensor("wo", [128, 2 * 1024], BF16,
                                 kind="ExternalInput"),
            "bq": nc.dram_tensor("bq", [128, 2], F32, kind="ExternalInput"),
            "bk": nc.dram_tensor("bk", [128, 2], F32, kind="ExternalInput"),
            "nbias": nc.dram_tensor("nbias", [128, NKC], F32,
                                    kind="ExternalInput"),
            "mdiag": nc.dram_tensor("mdiag", [128, 2, 128], BF16,
                                    kind="ExternalInput"),
            "y": nc.dram_tensor("y", [S, H], BF16, kind="ExternalOutput"),
            "y2": nc.dram_tensor("y2", [512, H], BF16, kind="ExternalOutput"),
        }
        _emit(nc, d)
        nc.finalize()
        _NC_CACHE["nc"] = nc
    return _NC_CACHE["nc"]


def _bf16(a):
    import ml_dtypes
    return np.ascontiguousarray(a.astype(ml_dtypes.bfloat16))


def _chunked(w, ncols):
    """[H, ncols] -> [128, NCH*ncols] with chunk c of rows at cols c*ncols."""
    return np.ascontiguousarray(
        w.reshape(NCH, 128, ncols).transpose(1, 0, 2).reshape(128, NCH * ncols))


def _make_in_maps(batch, input_ids, W_Q, W_K, W_V, b_Q, b_K, W_O):
    mdiag = np.broadcast_to(np.triu(np.ones((128, 128), np.float32)),
                            (2, 128, 128)).transpose(1, 0, 2)
    mdiag = _bf16(np.ascontiguousarray(mdiag))
    in_maps = []
    for core in range(NCORE):
        b, g = divmod(core, 4)
        base = 256 * g  # first feature column of this core's 4 heads
        wqq = np.stack([_chunked(W_Q[:, base + 128 * p: base + 128 * (p + 1)], 128)
                        for p in range(NPAIR)])
        wkk = np.stack([_chunked(W_K[:, base + 128 * p: base + 128 * (p + 1)], 128)
                        for p in range(NPAIR)])
        wv = _chunked(W_V[:, base: base + 256], 256)
        wo = np.ascontiguousarray(
            W_O[base: base + 256, :].reshape(2, 128, H)
            .transpose(1, 0, 2).reshape(128, 2 * H))
        bq = np.stack([b_Q[base + 128 * p: base + 128 * (p + 1)]
                       for p in range(NPAIR)], axis=1)
        bk = np.stack([b_K[base + 128 * p: base + 128 * (p + 1)]
                       for p in range(NPAIR)], axis=1)
        keep = input_ids[b] != 0
        nbias = np.where(keep, 0.0, NEG_BIAS).astype(np.float32)
        nbias = np.ascontiguousarray(nbias.reshape(NKC, 128).T)
        xt = np.ascontiguousarray(batch[b].T)
        in_maps.append({
            "xt": _bf16(xt), "wqq": _bf16(wqq), "wkk": _bf16(wkk),
            "wv": _bf16(wv), "wo": _bf16(wo),
            "bq": np.ascontiguousarray(bq), "bk": np.ascontiguousarray(bk),
            "nbias": nbias, "mdiag": mdiag,
        })
    return in_maps


def _run(in_maps, **kwargs):
    nc = _get_nc()
    return run_bass_kernel_spmd(nc, in_maps, core_ids=list(range(NCORE)), **kwargs)


def kernel(batch, input_ids, W_Q, W_K, W_V, b_Q, b_K, b_V, W_O, b_O,
           _results_out=None, **run_kwargs):
    batch = np.asarray(batch, np.float32)
    input_ids = np.asarray(input_ids)
    W_Q, W_K, W_V = (np.asarray(a, np.float32) for a in (W_Q, W_K, W_V))
    b_Q, b_K, b_V = (np.asarray(a, np.float32) for a in (b_Q, b_K, b_V))
    W_O = np.asarray(W_O, np.float32)
    b_O = np.asarray(b_O, np.float32)

    in_maps = _make_in_maps(batch, input_ids, W_Q, W_K, W_V, b_Q, b_K, W_O)
    res = _run(in_maps, **run_kwargs)
    if _results_out is not None:
        _results_out.append(res)
    ys = [np.asarray(res.results[c]["y"], np.float32) for c in range(NCORE)]
    y2s = [np.asarray(res.results[c]["y2"], np.float32) for c in range(NCORE)]
    out = np.stack([sum(ys[4 * b: 4 * b + 4]) for b in range(B)], axis=0)
    for b in range(B):
        out[b, 3 * 512:] += sum(y2s[4 * b: 4 * b + 4])
    # exact fold: attn rows sum to 1, so the V bias passes through W_O
    bias = b_V @ W_O + b_O
    return (out + bias).astype(np.float32)


# revision 54
# speedup vs baseline: 1.0218x; 1.0218x over previous
"""Multi-head attention (B=2, S=2048, H=1024, NH=16, DK=DV=64) on 8 TRN2 cores.

Sharding: data-parallel over batch (2 groups of 4 cores) x tensor-parallel
over heads (4 heads per core).  Each core computes, for its batch sample and
its 4 heads:
    Q^T/K^T projections (features on partitions), V projection (natural),
    S^T = K @ Q^T per 128-key chunk (causal chunks only; the two heads of a
    pair run as concurrent row-tiled K=64 matmuls into one 2-bank PSUM tile),
    P^T = exp(S^T/8 + pad_bias)  (one ACTIVATE covers both heads),
    out^T = V_aug^T @ P^T  where V_aug = [V | ones] for even heads and
    [ones | V] for odd heads, so values and 1/denominator stay
    partition-aligned for both halves of attn^T,
    y_partial = attn^T.T @ W_O_rows   (row-sharded W_O).
Host sums the 4 bf16 partials per batch and adds b_V @ W_O + b_O (exact
fold of the V bias through the output projection).

The emission is hand-staged so the ACT engine (exp is the serial bottleneck,
~88us/core) starts ~12us in and never starves, while projection/output
matmuls fill the PE between attention chunks and keep the PE HAM-warm:

    S1   : pair-0 Q/K projections as an 8-bank PSUM wave (c-outer), paced by
           the x^T DMA stream
    S2-5 : per j: V-projection chunks for j's keys, then att(j, pair0)
    S6-7 : pair-1 Q/K projection groups woven between att(0..1, pair1)
    S8-9 : att(2..3, pair1) with W_O(0..2) woven into the chunk loops
    S10  : W_O(3)

Everything on the wide data path is bf16 (fp32 accumulation in PSUM).
Engine balance: PE matmuls only; ACT exps only; DVE does projection
writebacks, normalization, and y casts; GpSimd does the causal-diagonal
masks.  All PSUM pool scopes are arranged to stay within the 8 banks.
"""

import math
from contextlib import ExitStack

import numpy as np

import concourse.bass as bass
import concourse.mybir as mybir
from concourse import bacc
import concourse.tile as tile
from concourse.bass_utils import run_bass_kernel_spmd

F32 = mybir.dt.float32
BF16 = mybir.dt.bfloat16
EXP = mybir.ActivationFunctionType.Exp

B, S, H = 2, 2048, 1024
NH, DK, DV = 16, 64, 64
NCORE = 8
NCH = H // 128          # 8 contraction chunks over H
NJ = S // 512           # 4 query subtiles of 512
NKC = S // 128          # 16 key chunks
NPAIR = 2               # head pairs per core
SCALE = 1.0 / math.sqrt(DK)
NEG_BIAS = -30000.0     # exp(x + NEG_BIAS) == 0.0 in fp32 for any real score


def _emit(nc, d):
    with tile.TileContext(nc) as tc, ExitStack() as top:
        consts = top.enter_context(tc.tile_pool(name="consts", bufs=1))
        persist = top.enter_context(tc.tile_pool(name="persist", bufs=1))
        xtp = top.enter_context(tc.tile_pool(name="xtp", bufs=1))

        # ---- persistent activations ----
        qt_sb = []   # per pair: [128, S] bf16; rows 0:64 head A, 64:128 head B
        kt_sb = []
        attnT = []   # per pair: [128, S] bf16 normalized attn^T
        for p in range(NPAIR):
            qt_sb.append(persist.tile([128, S], BF16, tag=f"qt{p}", name=f"qt{p}sb"))
            kt_sb.append(persist.tile([128, S], BF16, tag=f"kt{p}", name=f"kt{p}sb"))
            attnT.append(persist.tile([128, S], BF16, tag=f"at{p}", name=f"at{p}sb"))
        # V_aug per head: [128 keys, NKC*128]; chunk t block is [V|ones] for
        # even heads, [ones|V] for odd heads.
        vaug = []
        for h in range(4):
            v = persist.tile([128, NKC * 128], BF16, tag=f"vaug{h}", name=f"vaug{h}sb")
            nc.vector.memset(v, 1.0)
            vaug.append(v)

        xt_sb = [xtp.tile([128, S], BF16, tag=f"xt{c}", name=f"xt{c}sb")
                 for c in range(NCH)]

        # Two DMA rings: sync carries pair-0 weights + the x^T stream (the
        # critical path to first matmul); the ACT ring carries everything
        # needed later, in parallel.
        wqq_sb = []
        wkk_sb = []
        for p in range(NPAIR):
            wqq_sb.append(consts.tile([128, NCH * 128], BF16, tag=f"wqq{p}",
                                      name=f"wqq{p}sb"))
            wkk_sb.append(consts.tile([128, NCH * 128], BF16, tag=f"wkk{p}",
                                      name=f"wkk{p}sb"))
        # sync ring: nothing but the x^T stream (the gate for every matmul);
        # scalar ring: all weights, pair-0 first.
        for c in range(NCH):
            nc.sync.dma_start(out=xt_sb[c], in_=d["xt"][c * 128:(c + 1) * 128, :])
        nc.scalar.dma_start(out=wqq_sb[0], in_=d["wqq"][0])
        nc.scalar.dma_start(out=wkk_sb[0], in_=d["wkk"][0])
        bq_sb = consts.tile([128, 2], F32, tag="bq", name="bqsb")
        nc.scalar.dma_start(out=bq_sb, in_=d["bq"][:])
        bk_sb = consts.tile([128, 2], F32, tag="bk", name="bksb")
        nc.scalar.dma_start(out=bk_sb, in_=d["bk"][:])
        nbias_sb = consts.tile([128, NKC], F32, tag="nbias", name="nbiassb")
        nc.scalar.dma_start(out=nbias_sb, in_=d["nbias"][:])
        wv_sb = consts.tile([128, NCH * 256], BF16, tag="wv", name="wvsb")
        nc.scalar.dma_start(out=wv_sb, in_=d["wv"][:])
        nc.scalar.dma_start(out=wqq_sb[1], in_=d["wqq"][1])
        nc.scalar.dma_start(out=wkk_sb[1], in_=d["wkk"][1])
        wo_sb = consts.tile([128, 2 * 1024], BF16, tag="wo", name="wosb")
        nc.scalar.dma_start(out=wo_sb, in_=d["wo"][:])
        mdiag_sb = consts.tile([128, 2, 128], BF16, tag="mdiag", name="mdiagsb")
        nc.gpsimd.dma_start(out=mdiag_sb, in_=d["mdiag"][:])

        # ---- S1: pair-0 Q/K projections, 8-bank wave paced by the xt DMA ----
        with tc.tile_pool(name="psqk8", bufs=1, space="PSUM") as psqk8:
            # HAM pre-warm: the PE is otherwise idle from ~1us (vaug memsets
            # done) until the first weights+x^T land (~8us).  Dependency-free
            # matmuls on the memset vaug data hold the PE busy through the
            # 3.4us HAM window so the DMA-paced projection wave starts at
            # 2.4GHz instead of 1.2GHz.  Results are never read; the wave's
            # first tile reuses this bank afterwards (WAR-ordered).
            wrm = psqk8.tile([128, 512], F32, tag="pp00", name="hamwarm")
            for r in range(20):
                nc.tensor.matmul(
                    wrm, vaug[0][:, 0:128], vaug[0][:, 0:512],
                    start=(r == 0), stop=(r == 19))
            pss = {}
            for qk in range(2):
                for j in range(NJ):
                    pss[qk, j] = psqk8.tile([128, 512], F32, tag=f"pp{qk}{j}",
                                            name=f"pp{qk}{j}")
            for c in range(NCH - 1):
                for qk, wsb in ((0, wqq_sb[0]), (1, wkk_sb[0])):
                    for j in range(NJ):
                        nc.tensor.matmul(
                            pss[qk, j],
                            wsb[:, c * 128:(c + 1) * 128],
                            xt_sb[c][:, j * 512:(j + 1) * 512],
                            start=(c == 0), stop=False,
                        )
            # last chunk tile-major with immediate writeback so the 8
            # writebacks pipeline with the remaining matmuls instead of
            # draining serially at the end (j-major: att(0,0) unblocks first)
            c = NCH - 1
            for j in range(NJ):
                for qk, (wsb, bsb, dst) in enumerate((
                        (wqq_sb[0], bq_sb, qt_sb[0]),
                        (wkk_sb[0], bk_sb, kt_sb[0]))):
                    nc.tensor.matmul(
                        pss[qk, j],
                        wsb[:, c * 128:(c + 1) * 128],
                        xt_sb[c][:, j * 512:(j + 1) * 512],
                        start=False, stop=True,
                    )
                    nc.scalar.activation(
                        dst[:, j * 512:(j + 1) * 512], pss[qk, j],
                        mybir.ActivationFunctionType.Identity,
                        bias=bsb[:, 0:1])

        with (
            tc.tile_pool(name="psab", bufs=2, space="PSUM") as psab,
            tc.tile_pool(name="pso", bufs=1, space="PSUM") as psop,
            tc.tile_pool(name="ptp", bufs=4) as ptp,
            tc.tile_pool(name="nrm", bufs=3) as nrm,
            tc.tile_pool(name="ysb", bufs=4) as ysb,
        ):
            def qkproj_group(pool, p, qk, j):
                """Pair-1 style single-tile projection group (c-inner)."""
                wsb = (wqq_sb, wkk_sb)[qk][p]
                bsb = (bq_sb, bk_sb)[qk]
                dst = (qt_sb, kt_sb)[qk][p]
                ps = pool.tile([128, 512], F32, tag="sc", name=f"qk{p}{qk}{j}")
                for c in range(NCH):
                    nc.tensor.matmul(
                        ps, wsb[:, c * 128:(c + 1) * 128],
                        xt_sb[c][:, j * 512:(j + 1) * 512],
                        start=(c == 0), stop=(c == NCH - 1),
                    )
                # writeback on ACT: it slots between exps right where the
                # dependent scores need it, with no DVE-queue latency
                nc.scalar.activation(
                    dst[:, j * 512:(j + 1) * 512], ps,
                    mybir.ActivationFunctionType.Identity,
                    bias=bsb[:, p:p + 1])

            def vproj(pool, t):
                ps = pool.tile([128, 256], F32, tag="v", name=f"psv{t}")
                for c in range(NCH):
                    nc.tensor.matmul(
                        ps, xt_sb[c][:, t * 128:(t + 1) * 128],
                        wv_sb[:, c * 256:(c + 1) * 256],
                        start=(c == 0), stop=(c == NCH - 1),
                    )
                for h in range(4):
                    off = t * 128 + (0 if h % 2 == 0 else 64)
                    nc.vector.tensor_copy(vaug[h][:, off:off + 64],
                                          ps[:, h * 64:(h + 1) * 64])

            def norm_cols(p, j, oA, oB, scr, rec, col0, col1, on_act):
                """Normalize columns [col0:col1) of the (j,p) tile.  Legal
                before the accumulation group ends: PV chunk t only writes
                columns 128t onward, so lower columns are final early."""
                csl = slice(col0, col1)
                jsl = slice(j * 512 + col0, j * 512 + col1)
                if on_act:
                    nc.scalar.activation(scr[64:128, csl], oA[64:128, csl],
                                         mybir.ActivationFunctionType.Copy)
                    nc.scalar.activation(scr[0:64, csl], oB[0:64, csl],
                                         mybir.ActivationFunctionType.Copy)
                else:
                    nc.vector.tensor_copy(scr[64:128, csl], oA[64:128, csl])
                    nc.vector.tensor_copy(scr[0:64, csl], oB[0:64, csl])
                nc.sync.dma_start(out=rec[0:64, csl], in_=scr[64:128, csl])
                nc.sync.dma_start(out=rec[64:128, csl], in_=scr[0:64, csl])
                nc.vector.reciprocal_approx_fast(out=rec[:, csl],
                                                 in_=rec[:, csl])
                nc.vector.tensor_mul(attnT[p][0:64, jsl], oA[0:64, csl],
                                     rec[0:64, csl])
                nc.vector.tensor_mul(attnT[p][64:128, jsl], oB[64:128, csl],
                                     rec[64:128, csl])

            def finish_norm(j, p, oA, oB, last):
                # normalize: head A denom at oA[64:128], head B denom at
                # oB[0:64]; one cross DMA each makes values/recip lane-aligned.
                jsl = slice(j * 512, (j + 1) * 512)
                scr = nrm.tile([128, 512], F32, tag="scr", name=f"scr{p}{j}")
                if last:
                    nc.scalar.activation(scr[64:128, :], oA[64:128, :],
                                         mybir.ActivationFunctionType.Copy)
                    nc.scalar.activation(scr[0:64, :], oB[0:64, :],
                                         mybir.ActivationFunctionType.Copy)
                else:
                    nc.vector.tensor_copy(scr[64:128, :], oA[64:128, :])
                    nc.vector.tensor_copy(scr[0:64, :], oB[0:64, :])
                rec = nrm.tile([128, 512], F32, tag="rec", name=f"rec{p}{j}")
                nc.sync.dma_start(out=rec[0:64, :], in_=scr[64:128, :])
                nc.sync.dma_start(out=rec[64:128, :], in_=scr[0:64, :])
                nc.vector.reciprocal_approx_fast(out=rec, in_=rec)
                nc.vector.tensor_mul(attnT[p][0:64, jsl], oA[0:64, :],
                                     rec[0:64, :])
                nc.vector.tensor_mul(attnT[p][64:128, jsl], oB[64:128, :],
                                     rec[64:128, :])

            def att(j, p, weave=None, last=False, stage_wo=None):
                """Attention for (j, pair p): scores/exp/mask/PV + normalize.
                weave: optional {chunk_index: fn} emitted after that chunk.
                last: put the denominator copies on ACT (idle at the tail)
                to shorten the final normalize->W_O critical path."""
                hA, hB = 2 * p, 2 * p + 1
                oA = psop.tile([128, 512], F32, tag="oA", bufs=1, name=f"oA{p}{j}")
                oB = psop.tile([128, 512], F32, tag="oB", bufs=1, name=f"oB{p}{j}")
                if stage_wo:
                    s_scr = nrm.tile([128, 512], F32, tag="scr", name=f"sscr{p}{j}")
                    s_rec = nrm.tile([128, 512], F32, tag="rec", name=f"srec{p}{j}")
                cmax = 4 * j + 3
                for c in range(cmax + 1):
                    t = c - 4 * j
                    fo = 128 * t if t > 0 else 0
                    w = 512 - fo
                    qsl = slice(j * 512 + fo, (j + 1) * 512)
                    ksl = slice(c * 128, (c + 1) * 128)
                    sAB = psab.tile([128, 2, 512], F32, tag="sAB",
                                    name=f"sAB{p}{j}{c}")
                    nc.tensor.matmul(
                        sAB[:, 0, :w], kt_sb[p][0:64, ksl],
                        qt_sb[p][0:64, qsl], start=True, stop=True)
                    nc.tensor.matmul(
                        sAB[:, 1, :w], kt_sb[p][64:128, ksl],
                        qt_sb[p][64:128, qsl], start=True, stop=True)
                    pAB = ptp.tile([128, 2, 512], BF16, tag="pAB",
                                   name=f"pAB{p}{j}{c}")
                    if w == 512:
                        nc.scalar.activation(pAB[:, :, :], sAB[:, :, :], EXP,
                                             bias=nbias_sb[:, c:c + 1],
                                             scale=SCALE)
                    else:
                        nc.scalar.activation(pAB[:, :, :w], sAB[:, :, :w], EXP,
                                             bias=nbias_sb[:, c:c + 1],
                                             scale=SCALE)
                    if t >= 0:
                        # diagonal 128x128 block: zero keys below the diagonal
                        nc.gpsimd.tensor_mul(pAB[:, :, 0:128],
                                             pAB[:, :, 0:128], mdiag_sb)
                    nc.tensor.matmul(
                        oA[:, fo:512], vaug[hA][:, ksl], pAB[:, 0, :w],
                        start=(c == 0), stop=(c == cmax))
                    nc.tensor.matmul(
                        oB[:, fo:512], vaug[hB][:, ksl], pAB[:, 1, :w],
                        start=(c == 0), stop=(c == cmax))
                    if weave and c in weave:
                        weave[c]()
                    if stage_wo and t >= 0:
                        # columns [128t:128t+128) got their last PV write at
                        # this chunk — normalize them and run their W_O now
                        norm_cols(p, j, oA, oB, s_scr, s_rec,
                                  128 * t, 128 * t + 128, on_act=(t == 3))
                        stage_wo(t)
                if stage_wo:
                    return
                finish_norm(j, p, oA, oB, last)

            def wo_tile(pool, q, half, cast_eng=None, pairs=(0, 1), ydram=None,
                        yrow=None):
                pf = pool.tile([128, 512], F32, tag="sc", name=f"pf{q}{half}")
                for i, p in enumerate(pairs):
                    nc.tensor.matmul(
                        pf, attnT[p][:, q * 128:(q + 1) * 128],
                        wo_sb[:, p * 1024 + half * 512:
                              p * 1024 + half * 512 + 512],
                        start=(i == 0), stop=(i == len(pairs) - 1),
                    )
                yt = ysb.tile([128, 512], BF16, tag="y", name=f"yt{q}{half}")
                if cast_eng == "scalar":
                    nc.scalar.activation(yt, pf,
                                         mybir.ActivationFunctionType.Copy)
                else:
                    nc.vector.tensor_copy(yt, pf)
                ydst = d["y"] if ydram is None else d[ydram]
                row = q * 128 if yrow is None else yrow
                nc.sync.dma_start(
                    out=ydst[row:row + 128, half * 512:(half + 1) * 512],
                    in_=yt)

            # ---- S2-S5: V projection woven into att(*, pair0) chunk loops ----
            with tc.tile_pool(name="psv", bufs=2, space="PSUM") as psv:
                for t in range(4):
                    vproj(psv, t)
                att(0, 0)
                for j in range(1, NJ):
                    att(j, 0, weave={i: (lambda t=4 * j + i: vproj(psv, t))
                                     for i in range(4)})

            # ---- S6-S10: pair-1 projections + W_O woven into att(*, pair1);
            # qk-group and W_O psum tiles share one 2-bank rotating pool ----
            with tc.tile_pool(name="ps2", bufs=2, space="PSUM") as ps2:
                def qkg(qk, j):
                    return lambda: qkproj_group(ps2, 1, qk, j)

                def wot(q, half, cast_eng=None):
                    return lambda: wo_tile(ps2, q, half, cast_eng)

                qkproj_group(ps2, 1, 0, 0)
                qkproj_group(ps2, 1, 1, 0)
                att(0, 1, weave={0: qkg(0, 1), 2: qkg(1, 1)})
                # WO tiles woven one per chunk, each a stage behind its
                # normalize so the PE never waits on the DVE chain
                att(1, 1, weave={1: wot(0, 0), 2: wot(0, 1), 3: qkg(0, 2),
                                 4: wot(1, 0), 5: qkg(1, 2), 6: wot(1, 1),
                                 7: wot(2, 0)})
                att(2, 1, weave={0: wot(2, 1), 1: wot(3, 0), 2: qkg(0, 3),
                                 3: wot(3, 1), 4: wot(4, 0), 6: qkg(1, 3),
                                 7: wot(4, 1), 8: wot(5, 0), 9: wot(5, 1),
                                 10: wot(6, 0), 11: wot(6, 1)})
                # att(3,1) also absorbs the pair-0 halves of W_O(3) (split
                # output: y2 holds attnT0@W_O0 for rows 1536:2048, summed on
                # the host), so the tail after the last normalize is only the
                # eight single-matmul pair-1 tiles.
                def wop0(q, half):
                    return lambda: wo_tile(ps2, q, half, None, pairs=(0,),
                                           ydram="y2", yrow=(q - 12) * 128)

                def two(f, g):
                    return lambda: (f(), g())

                att(3, 1, weave={0: wot(7, 0), 1: wot(7, 1),
                                 2: wop0(12, 0), 3: wop0(12, 1),
                                 4: wot(8, 0), 5: wot(8, 1),
                                 6: wop0(13, 0), 7: wot(9, 0),
                                 8: wot(9, 1), 9: wop0(13, 1),
                                 10: wot(10, 0), 11: wot(10, 1),
                                 12: two(wop0(14, 0), wop0(14, 1)),
                                 13: wot(11, 0), 14: wot(11, 1),
                                 15: two(wop0(15, 0), wop0(15, 1))},
                    last=True)
                for q in range(12, 16):
                    wo_tile(ps2, q, 0, "scalar", pairs=(1,))
                    wo_tile(ps2, q, 1, None, pairs=(1,))


_NC_CACHE = {}


def _get_nc():
    if "nc" not in _NC_CACHE:
        nc = bacc.Bacc(None, target_bir_lowering=False)
        d = {
            "xt": nc.dram_tensor("xt", [H, S], BF16, kind="ExternalInput"),
            "wqq": nc.dram_tensor("wqq", [NPAIR, 128, NCH * 128], BF16,
                                  kind="ExternalInput"),
            "wkk": nc.dram_tensor("wkk", [NPAIR, 128, NCH * 128], BF16,
                                  kind="ExternalInput"),
            "wv": nc.dram_tensor("wv", [128, NCH * 256], BF16,
                                 kind="ExternalInput"),
            "wo": nc.dram_tensor("wo", [128, 2 * 1024], BF16,
                                 kind="ExternalInput"),
            "bq": nc.dram_tensor("bq", [128, 2], F32, kind="ExternalInput"),
            "bk": nc.dram_tensor("bk", [128, 2], F32, kind="ExternalInput"),
            "nbias": nc.dram_tensor("nbias", [128, NKC], F32,
                                    kind="ExternalInput"),
            "mdiag": nc.dram_tensor("mdiag", [128, 2, 128], BF16,
                                    kind="ExternalInput"),
            "y": nc.dram_tensor("y", [S, H], BF16, kind="ExternalOutput"),
            "y2": nc.dram_tensor("y2", [512, H], BF16, kind="ExternalOutput"),
        }
        _emit(nc, d)
        nc.finalize()
        _NC_CACHE["nc"] = nc
    return _NC_CACHE["nc"]


def _bf16(a):
    import ml_dtypes
    return np.ascontiguousarray(a.astype(ml_dtypes.bfloat16))


def _chunked(w, ncols):
    """[H, ncols] -> [128, NCH*ncols] with chunk c of rows at cols c*ncols."""
    return np.ascontiguousarray(
        w.reshape(NCH, 128, ncols).transpose(1, 0, 2).reshape(128, NCH * ncols))


def _make_in_maps(batch, input_ids, W_Q, W_K, W_V, b_Q, b_K, W_O):
    mdiag = np.broadcast_to(np.triu(np.ones((128, 128), np.float32)),
                            (2, 128, 128)).transpose(1, 0, 2)
    mdiag = _bf16(np.ascontiguousarray(mdiag))
    in_maps = []
    for core in range(NCORE):
        b, g = divmod(core, 4)
        base = 256 * g  # first feature column of this core's 4 heads
        wqq = np.stack([_chunked(W_Q[:, base + 128 * p: base + 128 * (p + 1)], 128)
                        for p in range(NPAIR)])
        wkk = np.stack([_chunked(W_K[:, base + 128 * p: base + 128 * (p + 1)], 128)
                        for p in range(NPAIR)])
        wv = _chunked(W_V[:, base: base + 256], 256)
        wo = np.ascontiguousarray(
            W_O[base: base + 256, :].reshape(2, 128, H)
            .transpose(1, 0, 2).reshape(128, 2 * H))
        bq = np.stack([b_Q[base + 128 * p: base + 128 * (p + 1)]
                       for p in range(NPAIR)], axis=1)
        bk = np.stack([b_K[base + 128 * p: base + 128 * (p + 1)]
                       for p in range(NPAIR)], axis=1)
        keep = input_ids[b] != 0
        nbias = np.where(keep, 0.0, NEG_BIAS).astype(np.float32)
        nbias = np.ascontiguousarray(nbias.reshape(NKC, 128).T)
        xt = np.ascontiguousarray(batch[b].T)
        in_maps.append({
            "xt": _bf16(xt), "wqq": _bf16(wqq), "wkk": _bf16(wkk),
            "wv": _bf16(wv), "wo": _bf16(wo),
            "bq": np.ascontiguousarray(bq), "bk": np.ascontiguousarray(bk),
            "nbias": nbias, "mdiag": mdiag,
        })
    return in_maps


def _run(in_maps, **kwargs):
    nc = _get_nc()
    return run_bass_kernel_spmd(nc, in_maps, core_ids=list(range(NCORE)), **kwargs)


def kernel(batch, input_ids, W_Q, W_K, W_V, b_Q, b_K, b_V, W_O, b_O,
           _results_out=None, **run_kwargs):
    batch = np.asarray(batch, np.float32)
    input_ids = np.asarray(input_ids)
    W_Q, W_K, W_V = (np.asarray(a, np.float32) for a in (W_Q, W_K, W_V))
    b_Q, b_K, b_V = (np.asarray(a, np.float32) for a in (b_Q, b_K, b_V))
    W_O = np.asarray(W_O, np.float32)
    b_O = np.asarray(b_O, np.float32)

    in_maps = _make_in_maps(batch, input_ids, W_Q, W_K, W_V, b_Q, b_K, W_O)
    res = _run(in_maps, **run_kwargs)
    if _results_out is not None:
        _results_out.append(res)
    ys = [np.asarray(res.results[c]["y"], np.float32) for c in range(NCORE)]
    y2s = [np.asarray(res.results[c]["y2"], np.float32) for c in range(NCORE)]
    out = np.stack([sum(ys[4 * b: 4 * b + 4]) for b in range(B)], axis=0)
    for b in range(B):
        out[b, 3 * 512:] += sum(y2s[4 * b: 4 * b + 4])
    # exact fold: attn rows sum to 1, so the V bias passes through W_O
    bias = b_V @ W_O + b_O
    return (out + bias).astype(np.float32)
